# revision 55
# baseline (speedup 1.0000x reference)
"""Causal single-head attention (B=8, N=2048, D=H=1024, fp32) on 8 TRN2 cores.

Data-parallel: one batch element per NeuronCore. Mixed fp8/bf16 design tuned
to the measured TRN2 PE behavior (1 moving column per cycle regardless of
dtype; fp8 DoubleRow contracts 256/instruction = 2x bf16 MACs, with weight
loads fully overlapped).

Score reformulation (saves the whole K projection): with G = Wq @ Wk^T and
u = Wk @ bq precomputed on the host,

  score[q,k] = (x_q Wq + bq) . (x_k Wk) = x_q G x_k^T + x_k . u

so the kernel computes ONE fp8 DoubleRow projection XGt = (G^T)8 @ (x^T)8
instead of two (Q and K), and the scores matmul contracts the resident x8
tiles (stationary) against XGt (moving). The rank-1 bias term r = x . u rides
the exp() activation as a per-partition (per-key) bias. bk is dropped: it is
exactly softmax-invariant.

  XGt      = fp8(e4m3) DoubleRow projection from x8/G8 (G pre-scaled by 64)
  scores^T = fp8 DoubleRow x8 . XGt8 (single chain)
  p        = exp(scores * 2^-11 + r * 2^-11) evicted straight to bf16
  V        = plain bf16 projection (elementwise-accurate; fp8 V/AV measured
             over the error budget - V errors hit the output directly)
  out      = bf16 p @ V / rowsum + bv   (bias applied post-normalization -
             it commutes with the attention average)

Everything stays resident in SBUF (no DRAM spills). Rowsums ride the PE as
1-column matmuls against a ones vector; the softmax division is folded into
the output eviction as a per-partition scale.

DMA discipline (HW-measured): each dma_start costs ~2.3us serialized on its
issuing ring regardless of size, and column-sliced loads shatter into
sub-512B descriptors at a fraction of line rate. So every input class is ONE
fully-contiguous DMA from a host-staged layout, spread over the three issue
queues (Sync/Scalar HWDGE + GpSimd SWDGE) in need-order. A short chain of
warmup matmuls on a constant tile keeps the PE busy (and the HAM clock-gate
warm) while the first wave lands.
"""

import os
import sys
from contextlib import ExitStack

import numpy as np
import ml_dtypes

# The concourse/bass toolchain comes from the container's python path; fall
# back to the /opt copy when running outside the preconfigured interpreter.
try:
    import concourse.bacc as bacc
except ImportError:  # pragma: no cover
    sys.path.insert(0, "/opt/trn_rl_repo")
    import concourse.bacc as bacc

import concourse.mybir as mybir
from concourse.tile import TileContext
from concourse.bass_utils import run_bass_kernel_spmd

# bass_utils imports antenv.axon_hooks when BASS_TRACE is set; provide a stub
# so tracing degrades gracefully instead of crashing if the module is absent.
try:
    import antenv.axon_hooks  # noqa: F401
except ImportError:  # pragma: no cover
    import types

    _m = types.ModuleType("antenv.axon_hooks")
    _m._hook = None
    _m.set_axon_ntff_profile_hook = lambda h: setattr(_m, "_hook", h)
    _m.get_axon_ntff_profile_hook = lambda: _m._hook
    sys.modules["antenv.axon_hooks"] = _m

# The boot-time NTFF hook install degrades silently when the image's antenv
# lacks axon_hooks; re-attempt it against our stub so BASS_TRACE captures
# HW profiles. Harmless no-op when axon or the .so is absent.
try:  # pragma: no cover
    import antenv.axon_hooks as _ah

    if _ah.get_axon_ntff_profile_hook() is None:
        from trn_agent_boot.trn_boot import _ntff_profile_via_ctypes

        _hook = _ntff_profile_via_ctypes("/opt/axon/libaxon_pjrt.so")
        if _hook is not None:
            _ah.set_axon_ntff_profile_hook(_hook)
except Exception:
    pass

B, N, D, H = 8, 2048, 1024, 1024
P = 128
DP = D // (2 * P)    # 4 fp8 contraction pair-tiles (256 deep each)
DT = D // P          # 8 bf16 contraction tiles
HP = H // (2 * P)    # 4 h pair-tiles for the score contraction
NT = N // P          # 16 sequence tiles of 128
IT = N // 512        # 4 query tiles of 512
GS = 64.0            # G/u pre-scale: centers G*GS in e4m3's normal range
EXP_SCALE = 1.0 / (np.sqrt(float(H)) * GS)  # 2^-11

F32 = mybir.dt.float32
F8 = mybir.dt.float8e4
BF16 = mybir.dt.bfloat16
F8NP = ml_dtypes.float8_e4m3
BFNP = ml_dtypes.bfloat16
DR = mybir.MatmulPerfMode.DoubleRow

WARM_MMS = int(os.environ.get("ATTN_WARM_MMS", "48"))
# The bq.k score-bias term shifts each key's softmax weights coherently
# across all queries; dropping it measured 2.3e-2 max-rel (over the gate).
# Keep it: 64 N=1 DoubleRow matmuls, ~2us.
USE_R = bool(int(os.environ.get("ATTN_R", "1")))

LAST_RESULT = None  # BassKernelResults of the most recent kernel() call
_CACHE = {}


def build_program(warm_mms: int = WARM_MMS, use_r: bool = USE_R):
    nc = bacc.Bacc("TRN2", target_bir_lowering=False, debug=False)

    # Host-staged layouts, one contiguous region per DMA below.
    x8d = nc.dram_tensor("x8d", [4, P, DP, 2, 512], F8, kind="ExternalInput")
    xbd = nc.dram_tensor("xbd", [P, DT, N], BF16, kind="ExternalInput")
    g8ad = nc.dram_tensor("g8ad", [P, DP, 2, 256], F8, kind="ExternalInput")
    g8bd = nc.dram_tensor("g8bd", [P, DP, 2, 256], F8, kind="ExternalInput")
    g8cd = nc.dram_tensor("g8cd", [P, DP, 2, 512], F8, kind="ExternalInput")
    u8d = nc.dram_tensor("u8d", [P, 2, DP], F8, kind="ExternalInput")
    wvbd = nc.dram_tensor("wvbd", [P, DT, H], BF16, kind="ExternalInput")
    bvB = nc.dram_tensor("bvB", [P, H], F32, kind="ExternalInput")
    out = nc.dram_tensor("out", [N, H], F32, kind="ExternalOutput")

    Exp = mybir.ActivationFunctionType.Exp
    Copy = mybir.ActivationFunctionType.Copy

    with TileContext(nc) as tc:
        with ExitStack() as top:
            const = top.enter_context(tc.tile_pool(name="const", bufs=1))
            kqv = top.enter_context(tc.tile_pool(name="kqv", bufs=1))
            ps_s = top.enter_context(tc.tile_pool(name="pss", bufs=3, space="PSUM"))

            ones_bf = const.tile([P, 1], BF16, tag="ones")
            nc.vector.memset(ones_bf[:], 1.0)
            warm_bf = const.tile([P, P], BF16, tag="warm")
            nc.vector.memset(warm_bf[:], 0.0)
            bv_sb = const.tile([P, H], F32, tag="bv")
            rT = const.tile([P, NT], F32, tag="rT")  # per-key exp bias (r*2^-11)

            # x^T fp8 pair data stays resident for the whole kernel: moving
            # operand of the XGt projection AND stationary (key-side) operand
            # of the scores contraction. One tile per 512-query chunk, all
            # four d-pair blocks inside, so each load is a single DMA.
            xall = [kqv.tile([P, DP, 2, 512], F8, tag=f"xa{c}", name=f"xa{c}") for c in range(4)]
            qp = [kqv.tile([P, 2, N], F8, tag=f"qp{i}", name=f"qp{i}") for i in range(HP)]
            vt = [kqv.tile([P, H], BF16, tag=f"vt{j}", name=f"vt{j}") for j in range(NT)]

            def xsd(d, j):
                """Stationary [P, 2, 128] x8 pair-slice d for key tile j."""
                return xall[j >> 2][:, d, :, (j & 3) * P:((j & 3) + 1) * P]

            # HAM warmup: keep the PE busy on a constant tile while the first
            # DMA wave lands, so the clock-gate is at 8/8 when real matmuls
            # start and the head DMA latency is hidden behind PE activity.
            for w in range(warm_mms):
                wps = ps_s.tile([P, 512], F32, tag="ps")
                nc.tensor.matmul(wps[:, 0:P], warm_bf[:], warm_bf[:], start=True, stop=True)

            # ---------------- Phase 1: projections (XGt, then V + r) ----------------
            with ExitStack() as p1:
                xpool = p1.enter_context(tc.tile_pool(name="xp", bufs=1))
                wpool = p1.enter_context(tc.tile_pool(name="wp", bufs=1))
                ps1 = p1.enter_context(tc.tile_pool(name="ps1", bufs=5, space="PSUM"))

                xball = xpool.tile([P, DT, N], BF16, tag="xb", name="xball")
                g8a = wpool.tile([P, DP, 2, 256], F8, tag="ga", name="g8a")
                g8b = wpool.tile([P, DP, 2, 256], F8, tag="gb", name="g8b")
                g8c = wpool.tile([P, DP, 2, 512], F8, tag="gc", name="g8c")
                u8 = wpool.tile([P, 2, DP], F8, tag="u8", name="u8")
                wvball = wpool.tile([P, DT, H], BF16, tag="wvb", name="wvball")

                def g8_slice(d, hb):
                    if hb < 2:
                        return g8a[:, d, :, hb * P:(hb + 1) * P]
                    if hb < 4:
                        return g8b[:, d, :, (hb - 2) * P:(hb - 1) * P]
                    return g8c[:, d, :, (hb - 4) * P:(hb - 3) * P]

                # One contiguous DMA per input class. All bulk rides ONE
                # SWDGE queue in strict need-order: queues share the same 16
                # SDMA engines and HBM bandwidth, so spreading bulk across
                # queues just makes the critical transfer compete with bulk
                # that is not needed for another 30us (measured: the g8 tail
                # landed at t=30us that way). FIFO on one queue = bandwidth
                # priority in program order. Only tiny/late items go on Sync.
                nc.sync.dma_start(g8a[:], g8ad.ap()[:])
                nc.gpsimd.dma_start(xall[0][:, 0:2], x8d.ap()[0, :, 0:2])
                nc.scalar.dma_start(xall[0][:, 2:DP], x8d.ap()[0, :, 2:DP])
                nc.gpsimd.dma_start(g8b[:], g8bd.ap()[:])
                nc.gpsimd.dma_start(xall[1][:], x8d.ap()[1])
                nc.gpsimd.dma_start(g8c[:], g8cd.ap()[:])
                nc.gpsimd.dma_start(xall[2][:], x8d.ap()[2])
                nc.gpsimd.dma_start(xall[3][:], x8d.ap()[3])
                nc.gpsimd.dma_start(wvball[:], wvbd.ap()[:])
                nc.gpsimd.dma_start(xball[:, :, 0:1024], xbd.ap()[:, :, 0:1024])
                nc.gpsimd.dma_start(xball[:, :, 1024:N], xbd.ap()[:, :, 1024:N])
                nc.sync.dma_start(bv_sb[:], bvB.ap()[:, :])
                if use_r:
                    nc.sync.dma_start(u8[:], u8d.ap()[:, :, :])

                # XGt[d', n] = sum_d G^T[d', d] x^T[d, n], fp8 DoubleRow.
                # Evicted straight to fp8 (values ~N(0, 21^2), well inside
                # e4m3's +-240 range).
                for nch in range(4):
                    cs = slice(nch * 512, (nch + 1) * 512)
                    for hb in range(8):
                        ps = ps1.tile([P, 512], F32, tag="ps")
                        for d in range(DP):
                            nc.tensor.matmul(
                                ps[:],
                                g8_slice(d, hb),
                                xall[nch][:, d],
                                start=(d == 0),
                                stop=(d == DP - 1),
                                perf_mode=DR,
                            )
                        dst = qp[hb >> 1][:, hb & 1, cs]
                        if hb & 1:
                            nc.vector.tensor_copy(dst, ps[:])
                        else:
                            nc.scalar.activation(dst, ps[:], Copy)

                # --- V = x @ Wv + bv in bf16, kept resident. Folding bv here
                # is exact: sum(p*(v+bv))/sum(p) == sum(p*v)/sum(p) + bv, and
                # it keeps the output eviction a pure scale+DMA.
                # The rank-1 score-bias column r[k] = x_k . u rides along as
                # one 4-matmul [P,1] DoubleRow group per key tile, reusing the
                # x8 stationaries; evicted pre-scaled by 2^-11 for the exp.
                for nb in range(NT):
                    ns = slice(nb * P, (nb + 1) * P)
                    for hch in range(2):
                        hs = slice(hch * 512, (hch + 1) * 512)
                        ps = ps1.tile([P, 512], F32, tag="ps")
                        for d in range(DT):
                            nc.tensor.matmul(
                                ps[:],
                                xball[:, d, ns],
                                wvball[:, d, hs],
                                start=(d == 0),
                                stop=(d == DT - 1),
                            )
                        nc.vector.tensor_add(vt[nb][:, hs], ps[:], bv_sb[:, hs])
                    if use_r:
                        # prr rides the ps1 rotation: the WAR dependency on
                        # the V psum group 1 back pins these matmuls into the
                        # V region (the scheduler otherwise hoists them into
                        # the XGt stream, where their DoubleRow weight
                        # reloads fragment the projection).
                        prr = ps1.tile([P, 1], F32, tag="ps")
                        for d in range(DP):
                            nc.tensor.matmul(
                                prr[:],
                                xsd(d, nb),
                                u8[:, :, d:d + 1],
                                start=(d == 0),
                                stop=(d == DP - 1),
                                perf_mode=DR,
                            )
                        nc.scalar.activation(rT[:, nb:nb + 1], prr[:], Copy, scale=float(EXP_SCALE))

            # ---------------- Phase 2: attention ----------------
            with ExitStack() as p2:
                pt_pool = p2.enter_context(tc.tile_pool(name="pt", bufs=1))
                sm = p2.enter_context(tc.tile_pool(name="sm", bufs=4))
                op_pool = p2.enter_context(tc.tile_pool(name="op", bufs=3))
                ps_av = p2.enter_context(tc.tile_pool(name="psav", bufs=3, space="PSUM"))
                ps_rs = p2.enter_context(tc.tile_pool(name="psrs", bufs=2, space="PSUM"))

                pt = [pt_pool.tile([P, 512], BF16, tag=f"pt{j}", name=f"pt{j}") for j in range(NT)]

                for t in range(IT):
                    i0 = 512 * t
                    jmax = 4 * t + 3

                    # scores^T [key j, query i] -> exp -> bf16 p, causal mask
                    # on the diagonal tiles. Columns below the diagonal cut c
                    # are never read by this t's AV matmuls.
                    for j in range(jmax + 1):
                        c = max(0, j * P - i0)
                        w = 512 - c
                        ps = ps_s.tile([P, 512], F32, tag="ps")
                        for hp_ in range(HP):
                            nc.tensor.matmul(
                                ps[:, 0:w],
                                xsd(hp_, j),
                                qp[hp_][:, :, i0 + c:i0 + 512],
                                start=(hp_ == 0),
                                stop=(hp_ == HP - 1),
                                perf_mode=DR,
                            )
                        if use_r:
                            nc.scalar.activation(
                                pt[j][:, c:512], ps[:, 0:w], Exp,
                                bias=rT[:, j:j + 1], scale=float(EXP_SCALE),
                            )
                        else:
                            nc.scalar.activation(
                                pt[j][:, c:512], ps[:, 0:w], Exp, scale=float(EXP_SCALE)
                            )
                        if c > 0 or j * P == i0:
                            # keep exp where key j*P+p <= query i0+c+f', else 0
                            nc.gpsimd.affine_select(
                                out=pt[j][:, c:512],
                                in_=pt[j][:, c:512],
                                compare_op=mybir.AluOpType.is_ge,
                                fill=0.0,
                                base=0,
                                channel_multiplier=-1,
                                pattern=[[1, w]],
                            )

                    # attn @ V, row-sums, normalize + bias on eviction
                    for s_ in range(4):
                        g = 4 * t + s_
                        final = g == NT - 1
                        qs = slice(s_ * P, (s_ + 1) * P)
                        pav = [ps_av.tile([P, 512], F32, tag="pav", name="pav") for _ in range(2)]
                        prs = ps_rs.tile([P, 1], F32, tag="prs")
                        recip = sm.tile([P, 1], F32, tag="recip")
                        if final:
                            # the last group sits on the kernel tail: run all
                            # rowsum matmuls first so the reciprocal is ready
                            # ~7us before the AV accumulation finishes, and
                            # the evictions can fire straight off the last MM.
                            for j in range(g + 1):
                                nc.tensor.matmul(
                                    prs[:], pt[j][:, qs], ones_bf[:],
                                    start=(j == 0), stop=(j == g),
                                )
                            nc.vector.reciprocal(recip[:], prs[:])
                        for j in range(g + 1):
                            lhsT = pt[j][:, qs]
                            for hch in range(2):
                                nc.tensor.matmul(
                                    pav[hch][:],
                                    lhsT,
                                    vt[j][:, hch * 512:(hch + 1) * 512],
                                    start=(j == 0),
                                    stop=(j == g),
                                )
                            if not final:
                                nc.tensor.matmul(
                                    prs[:], lhsT, ones_bf[:], start=(j == 0), stop=(j == g)
                                )
                        if not final:
                            nc.vector.reciprocal(recip[:], prs[:])
                        ot = op_pool.tile([P, H], F32, tag="ot")
                        rows = slice(i0 + s_ * P, i0 + (s_ + 1) * P)
                        if g < NT - 1:
                            # steady state: keep the Scalar engine free for the
                            # score exps (they gate the PE through the pss
                            # rotation) — both H-halves evict on Vector, both
                            # output DMAs issue from Sync.
                            nc.vector.tensor_scalar_mul(ot[:, 0:512], pav[0][:], recip[:])
                            nc.vector.tensor_scalar_mul(ot[:, 512:H], pav[1][:], recip[:])
                            nc.sync.dma_start(out.ap()[rows, 0:512], ot[:, 0:512])
                            nc.sync.dma_start(out.ap()[rows, 512:H], ot[:, 512:H])
                        else:
                            # final group sits on the kernel's critical tail:
                            # evict in 256-col chunks on alternating engines and
                            # fan the output DMAs across the issue queues so
                            # the ~600ns issues and completion latencies all
                            # overlap.
                            dma_eng = [nc.scalar, nc.sync, nc.gpsimd, nc.scalar]
                            for ch in range(4):
                                csl = slice(ch * 256, (ch + 1) * 256)
                                pv = pav[ch >> 1][:, (ch & 1) * 256:((ch & 1) + 1) * 256]
                                if ch & 1:
                                    nc.vector.tensor_scalar_mul(ot[:, csl], pv, recip[:])
                                else:
                                    nc.scalar.activation(ot[:, csl], pv, Copy, scale=recip[:])
                                dma_eng[ch].dma_start(out.ap()[rows, csl], ot[:, csl])

    nc.compile()
    return nc


def _get_program():
    key = (WARM_MMS, USE_R)
    if key not in _CACHE:
        _CACHE[key] = build_program(*key)
    return _CACHE[key]


def _pair_layout(mat):
    """[D, M] (already fp8) -> [DP, P, 2, M] pair layout, contiguous."""
    d, m_ = mat.shape
    return np.ascontiguousarray(mat.reshape(DP, 2, P, m_).transpose(0, 2, 1, 3))


def prep_inputs(x, Wq, bq, Wk, bk, Wv, bv):
    x = np.asarray(x, dtype=np.float32)
    Wq = np.asarray(Wq, dtype=np.float64)
    Wk = np.asarray(Wk, dtype=np.float64)
    Wv = np.asarray(Wv, dtype=np.float32)
    bq = np.asarray(bq, dtype=np.float64)
    bv = np.asarray(bv, dtype=np.float32)

    # score = x G x^T + x.u with G = Wq Wk^T, u = Wk bq (bk is softmax-
    # invariant and dropped). G rows are the contraction (d) pair dim.
    G = (Wq @ Wk.T) * GS
    u = (Wk @ bq) * GS
    g8_l = _pair_layout(G.astype(np.float32).astype(F8NP))  # [DP, P, 2, H]
    g8a_l = np.ascontiguousarray(g8_l[:, :, :, 0:256].transpose(1, 0, 2, 3))
    g8b_l = np.ascontiguousarray(g8_l[:, :, :, 256:512].transpose(1, 0, 2, 3))
    g8c_l = np.ascontiguousarray(g8_l[:, :, :, 512:H].transpose(1, 0, 2, 3))
    u8_l = np.ascontiguousarray(
        u.astype(np.float32).astype(F8NP).reshape(DP, 2, P).transpose(2, 1, 0)
    )
    wvb_l = np.ascontiguousarray(Wv.astype(BFNP).reshape(DT, P, H).transpose(1, 0, 2))
    bvB_h = np.ascontiguousarray(np.broadcast_to(bv, (P, H))).astype(np.float32)

    in_maps = []
    for b in range(B):
        xb_ = x[b]
        x8 = xb_.astype(F8NP)
        # [DP, P, 2, N] pair layout -> chunk-major [4, P, DP, 2, 512]
        x8p = _pair_layout(np.ascontiguousarray(x8.T))
        m = {
            "x8d": np.ascontiguousarray(
                x8p.reshape(DP, P, 2, 4, 512).transpose(3, 1, 0, 2, 4)
            ),
            "xbd": np.ascontiguousarray(
                xb_.T.astype(BFNP).reshape(DT, P, N).transpose(1, 0, 2)
            ),
            "g8ad": g8a_l,
            "g8bd": g8b_l,
            "g8cd": g8c_l,
            "u8d": u8_l,
            "wvbd": wvb_l,
            "bvB": bvB_h,
        }
        in_maps.append(m)
    return in_maps


def kernel(x, Wq, bq, Wk, bk, Wv, bv):
    global LAST_RESULT
    nc = _get_program()
    in_maps = prep_inputs(x, Wq, bq, Wk, bk, Wv, bv)
    res = run_bass_kernel_spmd(nc, in_maps, core_ids=list(range(B)))
    LAST_RESULT = res
    return np.stack([res.results[b]["out"] for b in range(B)], axis=0)
